# revision 13
# baseline (speedup 1.0000x reference)
"""AttCML distributed Bass kernel for 8 TRN2 NeuronCores.

Sharding: data-parallel over the batch dim (16384 / 8 = 2048 per core).

The on-device toolchain here has no usable wide-index row gather
(indirect DMA is not lowered by this walrus pipeline; the Q7 dma_gather
ucode is int16-indexed), so kernel() performs the embedding-row lookup
host-side and ships packed per-core tensors; all attention compute
(scores, exp/normalize, weighted pooling, distances) runs on device.

Key device-side trick: the reference masks prefs at position >= n+1.
Host fills those slots with the zero row, so exp(w)=1 there; the kernel
subtracts the pad count from the softmax denominator and the zero rows
contribute nothing to the weighted sum — exact semantics, no mask pass.

Per-core batch layout: batch element b_local = p * NT + t  (p = SBUF
partition 0..127, t = batch tile 0..NT-1).  The host unscrambles at the end.
"""

import numpy as np
from contextlib import ExitStack

try:
    import concourse  # noqa: F401
except ImportError:  # pragma: no cover
    import sys

    for _p in ("/opt/trn_rl_repo", "/root/.axon_site/_ro/trn_rl_repo"):
        if _p not in sys.path:
            sys.path.insert(0, _p)

import concourse.bacc as bacc
import concourse.tile as tile
from concourse import mybir
from concourse.bass_utils import run_bass_kernel_spmd

F32 = mybir.dt.float32
BF16 = mybir.dt.bfloat16
ALU = mybir.AluOpType
AXIS = mybir.AxisListType
ACTF = mybir.ActivationFunctionType

D = 128          # embedding dim
P = 50           # prefs per batch element
N_CORES = 8
B = 16384
BC = B // N_CORES  # 2048 batch per core
PB = 128           # batch tile = one SBUF partition set


def build_bass(bc: int = BC, cap=None):
    """Build the single-core Bass program.

    cap: per-tile pref-slot capacities (tuple of nt ints). The host assigns
    batch elements to tiles so element with n_prefs=n goes to a tile with
    cap >= n+1; slots beyond the element's n+1 hold the zero row.
    """
    nt = bc // PB
    if cap is None:
        cap = (P,) * nt
    assert len(cap) == nt
    offs = [0]
    for c in cap:
        offs.append(offs[-1] + c)
    ctot = offs[-1]

    nc = bacc.Bacc(
        "TRN2",
        target_bir_lowering=False,
        debug=False,
        enable_asserts=False,
        num_devices=N_CORES,
    )

    # pref rows, bf16, host-packed: [PB, sum(cap), D]
    pref_in = nc.declare_dram_parameter("pref", [PB, ctot * D], BF16, isOutput=False)
    # u/p/n vectors, f32: [PB, nt, D] each (tile-major in free dim)
    u_in = nc.declare_dram_parameter("uvec", [PB, nt * D], F32, isOutput=False)
    p_in = nc.declare_dram_parameter("pvec", [PB, nt * D], F32, isOutput=False)
    n_in = nc.declare_dram_parameter("nvec", [PB, nt * D], F32, isOutput=False)
    # pad counts (cap[t] - (n_b+1)) as f32: [PB, nt]
    padc_in = nc.declare_dram_parameter("padc", [PB, nt], F32, isOutput=False)
    out = nc.declare_dram_parameter("out", [PB, 2 * nt], F32, isOutput=True)

    with tile.TileContext(nc) as tc, ExitStack() as ctx:
        consts = ctx.enter_context(tc.tile_pool(name="consts", bufs=1))
        pref_pool = ctx.enter_context(tc.tile_pool(name="pref", bufs=2))
        tmp_pool = ctx.enter_context(tc.tile_pool(name="tmp", bufs=2))
        vec_pool = ctx.enter_context(tc.tile_pool(name="vec", bufs=2))
        small_pool = ctx.enter_context(tc.tile_pool(name="small", bufs=3))

        padc = consts.tile([PB, nt], F32)
        nc.sync.dma_start(padc[:], padc_in[:])
        res = consts.tile([PB, 2 * nt], F32)

        for t in range(nt):
            C = cap[t]
            L = C * D
            pref = pref_pool.tile([PB, L], BF16, tag="pref")
            nc.sync.dma_start(pref[:], pref_in[:, offs[t] * D : offs[t + 1] * D])
            pref3 = pref[:].rearrange("p (j d) -> p j d", d=D)

            u_t = vec_pool.tile([PB, D], F32, tag="u")
            nc.sync.dma_start(u_t[:], u_in[:, t * D : (t + 1) * D])
            p_t = vec_pool.tile([PB, D], F32, tag="pv")
            nc.sync.dma_start(p_t[:], p_in[:, t * D : (t + 1) * D])
            n_t = vec_pool.tile([PB, D], F32, tag="nv")
            nc.sync.dma_start(n_t[:], n_in[:, t * D : (t + 1) * D])

            # bf16 copies of targets for the 2x DVE mode in stage A
            p_b = vec_pool.tile([PB, D], BF16, tag="pb")
            nc.scalar.copy(p_b[:], p_t[:])
            n_b = vec_pool.tile([PB, D], BF16, tag="nb")
            nc.scalar.copy(n_b[:], n_t[:])

            for s, (tgt_b, tgt_f) in enumerate(((p_b, p_t), (n_b, n_t))):
                # ---- stage A: w[b, j] = pref[b, j, :] . tgt[b, :] ----
                tmp = tmp_pool.tile([PB, L], BF16, tag="tmp")
                tmp3 = tmp[:].rearrange("p (j d) -> p j d", d=D)
                tgt_bc = (
                    tgt_b[:]
                    .rearrange("p (o d) -> p o d", o=1)
                    .to_broadcast([PB, C, D])
                )
                nc.vector.tensor_tensor(out=tmp3, in0=pref3, in1=tgt_bc, op=ALU.mult)
                w = small_pool.tile([PB, P], F32, tag="w")
                nc.vector.tensor_reduce(
                    out=w[:, :C], in_=tmp3, axis=AXIS.X, op=ALU.add
                )

                # ---- stage B: att = exp(w) / (sum - padcount) ----
                e = small_pool.tile([PB, P], F32, tag="e")
                nc.scalar.activation(e[:, :C], w[:, :C], ACTF.Exp)
                ssum = small_pool.tile([PB, 1], F32, tag="ssum")
                nc.vector.tensor_reduce(
                    ssum[:], e[:, :C].rearrange("p (o j) -> p o j", o=1),
                    axis=AXIS.X, op=ALU.add,
                )
                scor = small_pool.tile([PB, 1], F32, tag="scor")
                nc.vector.tensor_tensor(
                    scor[:], ssum[:], padc[:, t : t + 1], op=ALU.subtract
                )
                rs = small_pool.tile([PB, 1], F32, tag="rs")
                nc.vector.reciprocal(rs[:], scor[:])
                att = small_pool.tile([PB, P], BF16, tag="att")
                nc.vector.tensor_scalar_mul(att[:, :C], e[:, :C], rs[:])

                # ---- stage C: r[b, :] = sum_j att[b, j] * pref[b, j, :] ----
                # att broadcast along d: expand on ACT, multiply 2x on DVE.
                attx = tmp_pool.tile([PB, L], BF16, tag="attx")
                attx3 = attx[:].rearrange("p (j d) -> p j d", d=D)
                att_bc = (
                    att[:, :C]
                    .rearrange("p (j o) -> p j o", o=1)
                    .to_broadcast([PB, C, D])
                )
                nc.scalar.copy(attx3, att_bc)
                tmp2 = tmp_pool.tile([PB, L], BF16, tag="tmp2")
                tmp23 = tmp2[:].rearrange("p (j d) -> p j d", d=D)
                nc.gpsimd.tensor_tensor(
                    out=tmp23, in0=pref3, in1=attx3, op=ALU.mult
                )
                r = vec_pool.tile([PB, D], F32, tag="r")
                nc.vector.tensor_reduce(
                    r[:],
                    tmp2[:].rearrange("p (j d) -> p d j", d=D),
                    axis=AXIS.X,
                    op=ALU.add,
                )

                # ---- distances: ||u + r - tgt||^2 ----
                du = vec_pool.tile([PB, D], F32, tag="du")
                nc.vector.tensor_sub(du[:], u_t[:], tgt_f[:])
                diff = vec_pool.tile([PB, D], F32, tag="diff")
                nc.vector.tensor_add(diff[:], r[:], du[:])
                sq = vec_pool.tile([PB, D], F32, tag="sq")
                nc.scalar.square(sq[:], diff[:])
                nc.vector.tensor_reduce(
                    out=res[:, s * nt + t : s * nt + t + 1],
                    in_=sq[:],
                    axis=AXIS.X,
                    op=ALU.add,
                )

        nc.sync.dma_start(out[:], res[:])

    nc.compile()
    return nc


_CACHE: dict = {}


def _get_bass(bc: int, cap: tuple):
    key = (bc, cap)
    if key not in _CACHE:
        _CACHE[key] = build_bass(bc, cap)
    return _CACHE[key]


def prep_core(user_emb, ctx_item_bf16, ctx_item, user_ids, pos_ids, neg_ids,
              pref_ids, n_prefs, cap, order):
    """Build one core's input map.

    order: [bc] permutation; element order[p * nt + t] is placed at
    partition p, tile t.  Host guarantees n_prefs[order[p*nt+t]] + 1 <= cap[t].
    """
    bc = order.shape[0]
    nt = bc // PB
    offs = np.concatenate([[0], np.cumsum(cap)]).astype(np.int64)
    ctot = int(offs[-1])

    ob = order.reshape(PB, nt)
    n1 = (n_prefs[ob] + 1.0).astype(np.float32)  # [PB, nt] valid counts

    pref = np.zeros((PB, ctot, D), dtype=ctx_item_bf16.dtype)
    for t in range(nt):
        C = int(cap[t])
        ids_t = pref_ids[ob[:, t], :C].copy()  # [PB, C]
        # mask-drop: slots >= n+1 -> zero row
        slot = np.arange(C)[None, :]
        ids_t[slot >= n1[:, t : t + 1]] = ctx_item_bf16.shape[0] - 1
        pref[:, offs[t] : offs[t + 1], :] = ctx_item_bf16[ids_t]

    uvec = user_emb[user_ids[ob].reshape(-1)].reshape(PB, nt * D)
    pvec = ctx_item[pos_ids[ob].reshape(-1)].reshape(PB, nt * D)
    nvec = ctx_item[neg_ids[ob].reshape(-1)].reshape(PB, nt * D)
    padc = (np.asarray(cap, np.float32)[None, :] - n1).astype(np.float32)

    return {
        "pref": np.ascontiguousarray(pref.reshape(PB, ctot * D)),
        "uvec": np.ascontiguousarray(uvec.astype(np.float32)),
        "pvec": np.ascontiguousarray(pvec.astype(np.float32)),
        "nvec": np.ascontiguousarray(nvec.astype(np.float32)),
        "padc": padc,
    }


def plan_order(n_prefs_core, cap):
    """Assign the core's bc elements to (partition, tile) slots so each
    element lands in a tile with cap >= n+1. Returns order [bc] or None."""
    bc = n_prefs_core.shape[0]
    nt = bc // PB
    idx = np.argsort(n_prefs_core, kind="stable")  # ascending n
    order = np.empty(bc, dtype=np.int64)
    # tiles sorted by capacity ascending; fill smallest-cap tiles with
    # smallest-n elements
    tile_order = np.argsort(np.asarray(cap), kind="stable")
    ok = True
    pos = 0
    for t in tile_order:
        members = idx[pos : pos + PB]
        if (n_prefs_core[members] + 1 > cap[t]).any():
            ok = False
        order[t::nt] = members  # partition p gets members[p] at tile t
        pos += PB
    if not ok:
        return None
    return order


# fixed capacity schedule (quantiles of Uniform{1..49} n_prefs + slack),
# independent of the data; falls back to all-50 if infeasible.
def default_caps(nt):
    qs = [min(P, int(np.ceil(2 + 48.0 * (i + 1) / nt)) + 4) for i in range(nt)]
    return tuple(qs)


def kernel(user_emb, item_emb, user_ids, pos_ids, neg_ids, pref_ids, n_prefs,
           _trace=False):
    user_emb = np.ascontiguousarray(np.asarray(user_emb, np.float32))
    item_emb = np.asarray(item_emb, np.float32)
    ctx_item = np.concatenate([item_emb, np.zeros((1, D), np.float32)], axis=0)
    # bf16 copy of the (ctx) item table for pref rows
    import ml_dtypes

    ctx_item_bf16 = ctx_item.astype(ml_dtypes.bfloat16)

    user_ids = np.asarray(user_ids)
    pos_ids = np.asarray(pos_ids)
    neg_ids = np.asarray(neg_ids)
    pref_ids = np.asarray(pref_ids)
    n_prefs = np.asarray(n_prefs, np.float32)

    nt = BC // PB
    cap = default_caps(nt)

    orders = []
    feasible = True
    for c in range(N_CORES):
        sl = slice(c * BC, (c + 1) * BC)
        o = plan_order(n_prefs[sl], cap)
        if o is None:
            feasible = False
            break
        orders.append(o)
    if not feasible:
        cap = (P,) * nt
        orders = [plan_order(n_prefs[c * BC : (c + 1) * BC], cap) for c in range(N_CORES)]

    nc = _get_bass(BC, cap)

    in_maps = []
    for c in range(N_CORES):
        sl = slice(c * BC, (c + 1) * BC)
        in_maps.append(
            prep_core(
                user_emb,
                ctx_item_bf16,
                ctx_item,
                user_ids[sl],
                pos_ids[sl],
                neg_ids[sl],
                pref_ids[sl],
                n_prefs[sl],
                cap,
                orders[c],
            )
        )

    res = run_bass_kernel_spmd(
        nc, in_maps, core_ids=list(range(N_CORES)), trace=_trace
    )

    out = np.empty((2, B), dtype=np.float32)
    for c in range(N_CORES):
        r = np.asarray(res.results[c]["out"])  # [PB, 2*nt]
        r = r.reshape(PB, 2, nt)  # [p, s, t]
        flat = r.transpose(1, 0, 2).reshape(2, BC)  # [(s), p*nt+t]
        out[:, c * BC : (c + 1) * BC][:, orders[c]] = flat
    if _trace:
        return out, res
    return out
